# revision 8
# baseline (speedup 1.0000x reference)
"""4-D average pool (kernel=2, stride=2) over [2,16,32,32,32,32] f32, on 8 NeuronCores.

Strategy: data-parallel over the 32 (b,c) slices -> 4 slices per core; the
per-core input is a contiguous [4096, 1024] f32 block (rows = (slice,d1,d2),
cols = (d3,d4)).

DMA ground rules (measured): only big CONTIGUOUS loads on a single HWDGE
ring sustain the ~383 GB/s per-core HBM rate under 8-core load; scattered
8 KiB gathers run ~2x slower and small (<=512 KiB) DMAs serialize on the
ring at ~2 us each.  So loads are contiguous [256q]-row blocks on the SP
ring with partition p = row-pair index (p = (row>>1) & 127, 8 KiB
descriptors).  That puts d2's low bit in the FREE dim -- DVE pools the d4,
d3, d2 pairs with three halving adds -- and leaves only d1's low bit in the
partition dim (p bit 4).  One bf16 matmul with a constant [128, 64] pairing
matrix (1/16 scale folded in, exact in bf16) pools it at ~1 ns/col -- 4x
cheaper than the f32 [128->32] alternative -- then ScalarE copies PSUM ->
SBUF f32 and the ACT-ring store writes y[64u : 64u+64q] contiguously.

The whole 16 MiB shard stays SBUF-resident so no load ever waits; DVE runs
~45% occupied and PE ~10%, so the gapless 43.8 us load stream is the
critical path.  The loads taper to two 1 MiB (256-row) chunks so only a
~3 us add/matmul/store chain trails the final byte (vs ~10 us for the f32
matmul pipeline), and the ~80-instruction kernel keeps the iram-load
preamble and event-semaphore teardown short.
"""

import sys

import numpy as np

if "/opt/trn_rl_repo" not in sys.path:
    sys.path.insert(0, "/opt/trn_rl_repo")

import concourse.bacc as bacc
import concourse.bass as bass
import concourse.tile as tile
from concourse import mybir
from concourse.bass_utils import run_bass_kernel_spmd

N_CORES = 8
SLICES_PER_CORE = 4  # 32 (b,c) slices / 8 cores
ROWS = SLICES_PER_CORE * 1024  # 4096
F32 = mybir.dt.float32
BF16 = mybir.dt.bfloat16
# chunk sizes in 256-row (1 MiB) units: 2 MiB bulk, 1 MiB tail taper
UNITS = [2, 2, 2, 2, 2, 2, 2, 1, 1]


def _build_pm() -> np.ndarray:
    # pm[p, o] = 1/16 iff o = (p>>5)*16 + (p&15): pools partition bit 4
    # (= d1 low bit) and applies the average scale (0.0625 is exact in bf16)
    import ml_dtypes

    pm = np.zeros((128, 64), np.float32)
    for p in range(128):
        pm[p, (p >> 5) * 16 + (p & 15)] = 1.0 / 16.0
    return pm.astype(ml_dtypes.bfloat16)


def build_nc() -> bass.Bass:
    # Bacc (not raw Bass): its compile() splits multi-sem sync waits into
    # event-semaphore instructions (TRN2 allows one wait per instruction).
    nc = bacc.Bacc()
    x = nc.dram_tensor("x", [ROWS, 1024], F32, kind="ExternalInput")
    pm = nc.dram_tensor("pm", [128, 64], BF16, kind="ExternalInput")
    y = nc.dram_tensor("y", [ROWS // 4, 256], F32, kind="ExternalOutput")

    with tile.TileContext(nc) as tc:
        with (
            tc.tile_pool(name="pmp", bufs=1) as pmp,
            # one slot per chunk -> the whole 16 MiB shard is SBUF-resident,
            # so load DMAs carry no waits and stream back-to-back
            tc.tile_pool(name="inp", bufs=len(UNITS)) as inp,
            tc.tile_pool(name="m1p", bufs=2) as m1p,
            tc.tile_pool(name="m2p", bufs=2) as m2p,
            tc.tile_pool(name="m3p", bufs=4) as m3p,
            tc.tile_pool(name="psp", bufs=4, space=bass.MemorySpace.PSUM) as psp,
            tc.tile_pool(name="obp", bufs=4) as obp,
        ):
            pm_t = pmp.tile([128, 64], BF16)

            B = 0
            for ci, q in enumerate(UNITS):
                # contiguous [256q, 1024] block; partition = row-pair index
                t = inp.tile([128, 2048 * q], F32, tag="t")
                src = x[256 * B : 256 * (B + q), :].rearrange(
                    "(q p r0) c -> p q (r0 c)", p=128, r0=2
                )
                nc.sync.dma_start(
                    t[:].rearrange("p (q c) -> p q c", q=q), src
                )
                if ci == 0:
                    # pm load after the first bulk DMA: only needed by the
                    # first matmul (~7 us in), off the critical path
                    nc.sync.dma_start(pm_t[:], pm[:])

                # free = (q, d2l 2, d3 32, d4 32).  Pool d2l FIRST: its
                # operands are contiguous 4 KiB blocks (cheapest DVE read
                # pattern), and the bf16 output doubles DVE rate for the
                # two interleaved levels that follow.
                tv = t[:].rearrange("p (q e2 c) -> p q e2 c", q=q, e2=2)
                m1 = m1p.tile([128, q * 1024], BF16, tag="m1")
                m1v = m1[:].rearrange("p (q c) -> p q c", q=q)
                nc.vector.tensor_add(m1v, tv[:, :, 0, :], tv[:, :, 1, :])

                # pool d4 pairs: free (a=(q,d3), o4 16, e4 2)
                a = q * 32
                v = m1[:].rearrange("p (a o4 e4) -> p a o4 e4", a=a, o4=16)
                m2 = m2p.tile([128, a * 16], BF16, tag="m2")
                m2v = m2[:].rearrange("p (a o4) -> p a o4", a=a)
                nc.vector.tensor_add(m2v, v[:, :, :, 0], v[:, :, :, 1])

                # pool d3 pairs: free (b=q, o3 16, e3 2, o4 16)
                w = m2[:].rearrange(
                    "p (b o3 e3 o4) -> p b o3 e3 o4", b=q, o3=16, e3=2
                )
                m3 = m3p.tile([128, q * 256], BF16, tag="m3")
                m3v = m3[:].rearrange("p (b o3 o4) -> p b o3 o4", b=q, o3=16)
                nc.vector.tensor_add(m3v, w[:, :, :, 0, :], w[:, :, :, 1, :])

                # pool the d1l partition pairs (+1/16 scale) in one matmul
                ps = psp.tile([64, q * 256], F32, tag="ps")
                nc.tensor.matmul(ps[:], pm_t[:], m3[:], start=True, stop=True)
                ob = obp.tile([64, q * 256], F32, tag="ob")
                nc.scalar.copy(ob[:], ps[:])

                # unit u's 64 output rows are y[64u : 64u+64] -- contiguous
                dst = y[64 * B : 64 * (B + q), :].rearrange(
                    "(q r) c -> r q c", r=64
                )
                nc.scalar.dma_start(dst, ob[:].rearrange("r (q c) -> r q c", q=q))
                B += q

    nc.compile()
    return nc


_NC_CACHE: bass.Bass | None = None


def kernel(nd_tensor: np.ndarray, _trace: bool = False):
    global _NC_CACHE
    x = np.ascontiguousarray(np.asarray(nd_tensor, dtype=np.float32)).reshape(
        32, 1024, 1024
    )
    if _NC_CACHE is None:
        _NC_CACHE = build_nc()
    nc = _NC_CACHE
    pm = _build_pm()

    in_maps = [
        {
            "x": np.ascontiguousarray(
                x[SLICES_PER_CORE * i : SLICES_PER_CORE * (i + 1)]
            ).reshape(ROWS, 1024),
            "pm": pm,
        }
        for i in range(N_CORES)
    ]
    res = run_bass_kernel_spmd(
        nc, in_maps, core_ids=list(range(N_CORES)), trace=_trace
    )
    out = np.stack([res.results[i]["y"] for i in range(N_CORES)])  # [8,1024,256]
    out = out.reshape(2, 16, 16, 16, 16, 16).astype(np.float32)
    if _trace:
        kernel.last_results = res
    return out


# revision 9
# speedup vs baseline: 1.0380x; 1.0380x over previous
"""4-D average pool (kernel=2, stride=2) over [2,16,32,32,32,32] f32, on 8 NeuronCores.

Strategy: data-parallel over the 32 (b,c) slices -> 4 slices per core; the
per-core input is a contiguous [4096, 1024] f32 block (rows = (slice,d1,d2),
cols = (d3,d4)).

DMA ground rules (measured): only big CONTIGUOUS loads on a single HWDGE
ring sustain the ~383 GB/s per-core HBM rate under 8-core load; scattered
8 KiB gathers run ~2x slower and small (<=512 KiB) DMAs serialize on the
ring at ~2 us each.  So loads are contiguous [256q]-row blocks on the SP
ring with partition p = row-pair index (p = (row>>1) & 127, 8 KiB
descriptors).  That puts d2's low bit in the FREE dim -- DVE pools the d4,
d3, d2 pairs with three halving adds -- and leaves only d1's low bit in the
partition dim (p bit 4).  One bf16 matmul with a constant [128, 64] pairing
matrix (1/16 scale folded in, exact in bf16) pools it at ~1 ns/col -- 4x
cheaper than the f32 [128->32] alternative -- then ScalarE copies PSUM ->
SBUF f32 and the ACT-ring store writes y[64u : 64u+64q] contiguously.

The whole 16 MiB shard stays SBUF-resident so no load ever waits; DVE runs
~45% occupied and PE ~10%, so the gapless 43.8 us load stream is the
critical path.  The loads taper to two 1 MiB (256-row) chunks so only a
~3 us add/matmul/store chain trails the final byte (vs ~10 us for the f32
matmul pipeline), and the ~80-instruction kernel keeps the iram-load
preamble and event-semaphore teardown short.
"""

import sys

import numpy as np

if "/opt/trn_rl_repo" not in sys.path:
    sys.path.insert(0, "/opt/trn_rl_repo")

import concourse.bacc as bacc
import concourse.bass as bass
import concourse.tile as tile
from concourse import mybir
from concourse.bass_utils import run_bass_kernel_spmd

N_CORES = 8
SLICES_PER_CORE = 4  # 32 (b,c) slices / 8 cores
ROWS = SLICES_PER_CORE * 1024  # 4096
F32 = mybir.dt.float32
BF16 = mybir.dt.bfloat16
# chunk sizes in 256-row (1 MiB) units: 2 MiB bulk, 1 MiB tail taper
UNITS = [2, 2, 2, 2, 2, 2, 1, 1, 1, 1]
# tail chunks whose adds run on GPSIMD so the final DVE chains don't stack:
# without this, chunks landing in the last ~8 us serialize ~3 chains on DVE
GP_CHUNKS = {6}


def _build_pm() -> np.ndarray:
    # pm[p, o] = 1/16 iff o = (p>>5)*16 + (p&15): pools partition bit 4
    # (= d1 low bit) and applies the average scale (0.0625 is exact in bf16)
    import ml_dtypes

    pm = np.zeros((128, 64), np.float32)
    for p in range(128):
        pm[p, (p >> 5) * 16 + (p & 15)] = 1.0 / 16.0
    return pm.astype(ml_dtypes.bfloat16)


def build_nc() -> bass.Bass:
    # Bacc (not raw Bass): its compile() splits multi-sem sync waits into
    # event-semaphore instructions (TRN2 allows one wait per instruction).
    nc = bacc.Bacc()
    x = nc.dram_tensor("x", [ROWS, 1024], F32, kind="ExternalInput")
    pm = nc.dram_tensor("pm", [128, 64], BF16, kind="ExternalInput")
    y = nc.dram_tensor("y", [ROWS // 4, 256], F32, kind="ExternalOutput")

    with tile.TileContext(nc) as tc:
        with (
            tc.tile_pool(name="pmp", bufs=1) as pmp,
            # one slot per chunk -> the whole 16 MiB shard is SBUF-resident,
            # so load DMAs carry no waits and stream back-to-back
            tc.tile_pool(name="inp", bufs=len(UNITS)) as inp,
            tc.tile_pool(name="m1p", bufs=2) as m1p,
            tc.tile_pool(name="m2p", bufs=2) as m2p,
            tc.tile_pool(name="m3p", bufs=4) as m3p,
            tc.tile_pool(name="psp", bufs=4, space=bass.MemorySpace.PSUM) as psp,
            tc.tile_pool(name="obp", bufs=4) as obp,
        ):
            pm_t = pmp.tile([128, 64], BF16)

            B = 0
            for ci, q in enumerate(UNITS):
                # contiguous [256q, 1024] block; partition = row-pair index
                t = inp.tile([128, 2048 * q], F32, tag="t")
                src = x[256 * B : 256 * (B + q), :].rearrange(
                    "(q p r0) c -> p q (r0 c)", p=128, r0=2
                )
                nc.sync.dma_start(
                    t[:].rearrange("p (q c) -> p q c", q=q), src
                )
                if ci == 0:
                    # pm load after the first bulk DMA: only needed by the
                    # first matmul (~7 us in), off the critical path
                    nc.sync.dma_start(pm_t[:], pm[:])

                # free = (q, d2l 2, d3 32, d4 32).  f32 intermediates:
                # bf16 DVE writes measurably throttle the concurrent load
                # stream (sub-word SBUF writes), and DVE cost is set by
                # output element count regardless of dtype or stride.
                eng = nc.gpsimd if ci in GP_CHUNKS else nc.vector
                a = q * 64
                v = t[:].rearrange("p (a o4 e4) -> p a o4 e4", a=a, o4=16)
                m1 = m1p.tile([128, a * 16], F32, tag="m1")
                m1v = m1[:].rearrange("p (a o4) -> p a o4", a=a)
                eng.tensor_add(m1v, v[:, :, :, 0], v[:, :, :, 1])

                b = q * 2
                w = m1[:].rearrange(
                    "p (b o3 e3 o4) -> p b o3 e3 o4", b=b, o3=16, e3=2
                )
                m2 = m2p.tile([128, b * 256], F32, tag="m2")
                m2v = m2[:].rearrange("p (b o3 o4) -> p b o3 o4", b=b, o3=16)
                eng.tensor_add(m2v, w[:, :, :, 0, :], w[:, :, :, 1, :])

                # pool d2l pairs, casting to bf16 for the cheap matmul
                z = m2[:].rearrange("p (qq e2 c) -> p qq e2 c", qq=q, e2=2)
                m3 = m3p.tile([128, q * 256], BF16, tag="m3")
                m3v = m3[:].rearrange("p (qq c) -> p qq c", qq=q)
                eng.tensor_add(m3v, z[:, :, 0, :], z[:, :, 1, :])

                # pool the d1l partition pairs (+1/16 scale) in one matmul
                ps = psp.tile([64, q * 256], F32, tag="ps")
                nc.tensor.matmul(ps[:], pm_t[:], m3[:], start=True, stop=True)
                ob = obp.tile([64, q * 256], F32, tag="ob")
                nc.scalar.copy(ob[:], ps[:])

                # unit u's 64 output rows are y[64u : 64u+64] -- contiguous
                dst = y[64 * B : 64 * (B + q), :].rearrange(
                    "(q r) c -> r q c", r=64
                )
                nc.scalar.dma_start(dst, ob[:].rearrange("r (q c) -> r q c", q=q))
                B += q

    nc.compile()
    return nc


_NC_CACHE: bass.Bass | None = None


def kernel(nd_tensor: np.ndarray, _trace: bool = False):
    global _NC_CACHE
    x = np.ascontiguousarray(np.asarray(nd_tensor, dtype=np.float32)).reshape(
        32, 1024, 1024
    )
    if _NC_CACHE is None:
        _NC_CACHE = build_nc()
    nc = _NC_CACHE
    pm = _build_pm()

    in_maps = [
        {
            "x": np.ascontiguousarray(
                x[SLICES_PER_CORE * i : SLICES_PER_CORE * (i + 1)]
            ).reshape(ROWS, 1024),
            "pm": pm,
        }
        for i in range(N_CORES)
    ]
    res = run_bass_kernel_spmd(
        nc, in_maps, core_ids=list(range(N_CORES)), trace=_trace
    )
    out = np.stack([res.results[i]["y"] for i in range(N_CORES)])  # [8,1024,256]
    out = out.reshape(2, 16, 16, 16, 16, 16).astype(np.float32)
    if _trace:
        kernel.last_results = res
    return out
